# revision 1
# baseline (speedup 1.0000x reference)
"""Trainium2 Bass kernel for nn_BlendHydroV1 (HBV + ExpHydro blend + gamma routing).

Strategy
--------
Shard the 4000 basins over 8 NeuronCores (500 basins/core). Per core the
500 basins x NMUL(2) = 1000 columns (padded to 1024) are laid out as
[128 partitions, 8 free] f32 tiles; column c -> partition c//8, lane c%8.

All parameter descaling, forcing-dependent precomputes (rain/snow split,
melt caps, refreeze caps) and routing weights are folded on the host and
shipped as DRAM arrays. On device:

  * EXP-model snow store s0 solved via `tensor_tensor_scan`
    (state' = max(state + (ps-mc), ps)) in 8 instructions total.
  * Sequential 730-step loop runs the remaining coupled recurrences
    (HBV sp/mw/sm/suz, EXP s1) with ~36 small DVE ops + 3 ACT ops/step.
    Per-step q components are written into big SBUF sequence buffers.
  * HBV lower zone (slz) solved post-loop via scans; q0/q1/q2/qspill/qb
    recovery, NMUL-blend and 15-tap gamma routing run as large-free-dim
    batched ops.

Self-contained: hardcodes all shapes; only needs `concourse` (+jax/axon)
from the environment.
"""
import numpy as np

S, G, NMUL, LENF = 730, 4000, 2, 15
NCORES = 8
GPC = G // NCORES            # basins per core (500)
BPAD = 512                   # padded basins per core
NCOL = BPAD * NMUL           # 1024 columns
NPART = 128
NL = NCOL // NPART           # 8 lanes per partition
U = 73                       # time steps per For_i iteration
NITER = S // U               # 10
NEARZERO = 1e-5
QSPAD = LENF - 1             # 14
QSW = S + QSPAD              # 744

f32 = np.float32
HBV_LB = np.array([1., 50., .05, .01, .001, .2, 0., 0., -2.5, .5, 0., 0.], f32)
HBV_UB = np.array([6., 1000., .9, .5, .2, 1., 10., 100., 2.5, 10., .1, .2], f32)
EXP_LB = np.array([0., 100., 10., 0., 0., -3.], f32)
EXP_UB = np.array([.1, 1500., 50., 5., 3., 0.], f32)

# param lane order in the `par` DRAM tensor (each 8 wide)
PAR_NAMES = ["cwh", "ifc", "beta", "fc", "ilpfc", "perc", "uzl", "k0",
             "k1c", "smax", "ismx", "nf", "qmx", "k2c", "k1"]
NPARAM = len(PAR_NAMES)      # 15
W4_OFF = NPARAM * NL         # 120
PAR_W = W4_OFF + LENF * 4    # 180


# --------------------------------------------------------------------------
# host-side preparation
# --------------------------------------------------------------------------

def _host_prepare(x, raw_phy_static):
    """Build per-core DRAM arrays. Returns list of dicts (one per core)."""
    x = np.ascontiguousarray(np.asarray(x, f32))
    raw = np.ascontiguousarray(np.asarray(raw_phy_static, f32))

    static = raw[:, :18 * NMUL].reshape(G, 18, NMUL)
    ph = (HBV_LB[None, :, None] + static[:, :12, :]
          * (HBV_UB - HBV_LB)[None, :, None]).astype(f32)
    pe = (EXP_LB[None, :, None] + static[:, 12:, :]
          * (EXP_UB - EXP_LB)[None, :, None]).astype(f32)

    def cols(a):      # [G, NMUL] -> [G*NMUL], col = g*2+m
        return np.ascontiguousarray(a).reshape(-1)

    beta, fc, k0, k1, k2, lp, perc, uzl, tt, cfmax, cfr, cwh = \
        [cols(ph[:, i, :]) for i in range(12)]
    fexp, smax, qmax, df, tmax, tmin = [cols(pe[:, i, :]) for i in range(6)]

    ifc = (1.0 / fc.astype(np.float64)).astype(f32)
    ilpfc = (1.0 / (lp.astype(np.float64) * fc.astype(np.float64))).astype(f32)
    ismx = (1.0 / smax.astype(np.float64)).astype(f32)
    ccfm = (cfr * cfmax).astype(f32)
    k1c = (f32(1) - k1).astype(f32)
    k2c = (f32(1) - k2).astype(f32)
    nf = (-fexp).astype(f32)

    P = x[:, :, 0]
    T = x[:, :, 1]
    PET = x[:, :, 2]
    Pc = np.repeat(P, NMUL, axis=1)
    Tc = np.repeat(T, NMUL, axis=1)
    PETc = np.repeat(PET, NMUL, axis=1)

    rain = np.where(Tc >= tt[None, :], Pc, f32(0)).astype(f32)
    snow = (Pc - rain).astype(f32)
    ru = np.maximum((cfmax[None, :] * (Tc - tt[None, :])).astype(f32), f32(0))
    rv = np.maximum((ccfm[None, :] * (tt[None, :] - Tc)).astype(f32), f32(0))
    ps = np.where(Tc <= tmin[None, :], Pc, f32(0)).astype(f32)
    mc = np.where(Tc > tmax[None, :],
                  (df[None, :] * (Tc - tmax[None, :])).astype(f32), f32(0))
    scan_c = (ps - mc).astype(f32)

    # routing weights (reference _uh_gamma in f32, scaled by 0.25)
    from scipy.special import gammaln
    route = raw[:, 18 * NMUL:]
    a = (route[:, 0] * f32(2.9)).astype(f32)
    b = (route[:, 1] * f32(6.5)).astype(f32)
    aa = (np.maximum(a, f32(0)) + f32(0.1)).astype(f32)
    th = (np.maximum(b, f32(0)) + f32(0.5)).astype(f32)
    tgrid = (np.arange(LENF, dtype=f32) + f32(0.5))
    logw = (-gammaln(aa.astype(np.float64)).astype(f32)[None, :]
            - (aa * np.log(th).astype(f32))[None, :]
            + np.outer(np.log(tgrid).astype(f32), (aa - f32(1)))
            - np.outer(tgrid, (1.0 / th.astype(np.float64)).astype(f32)))
    w = np.exp(logw.astype(f32)).astype(f32)
    w = (w / w.sum(0, keepdims=True)).astype(f32)          # [LENF, G]
    w4 = (w * f32(0.25)).astype(f32)

    params = dict(cwh=cwh, ifc=ifc, beta=beta, fc=fc, ilpfc=ilpfc, perc=perc,
                  uzl=uzl, k0=k0, k1c=k1c, smax=smax, ismx=ismx, nf=nf,
                  qmx=qmax, k2c=k2c, k1=k1)
    planes = [snow, ru, rv, rain, PETc]     # in-loop forcing order
    pre_planes = [scan_c, ps, Pc]           # C, B, PTOT (j-major per core)

    per_core = []
    for d in range(NCORES):
        c0, c1 = d * GPC * NMUL, (d + 1) * GPC * NMUL     # 1000 cols
        padw = NCOL - (c1 - c0)

        def shard(v):      # [..., cols] -> padded [... , NCOL]
            s = v[..., c0:c1]
            return np.pad(s, [(0, 0)] * (s.ndim - 1) + [(0, padw)], mode="edge")

        # par: [128, PAR_W]
        par = np.zeros((NPART, PAR_W), f32)
        for i, nm in enumerate(PAR_NAMES):
            par[:, i * NL:(i + 1) * NL] = shard(params[nm]).reshape(NPART, NL)
        wsh = np.pad(w4[:, d * GPC:(d + 1) * GPC],
                     [(0, 0), (0, BPAD - GPC)], mode="edge")  # [LENF, 512]
        # basin b = 4p + j'  ->  par[p, W4_OFF + k*4 + j']
        par[:, W4_OFF:] = wsh.reshape(LENF, NPART, 4).transpose(1, 0, 2).reshape(NPART, LENF * 4)

        # forc: [128, S*5*NL], slot ((t*5)+f)*NL + j
        fstk = np.stack([shard(pl) for pl in planes], axis=1)   # [S, 5, NCOL]
        forc = fstk.reshape(S * 5, NPART, NL).transpose(1, 0, 2).reshape(NPART, S * 5 * NL)

        # pre: [128, 3*NL*S]  (j-major: plane*NL*S + j*S + t)
        pstk = np.stack([shard(pl) for pl in pre_planes], axis=0)  # [3, S, NCOL]
        pre = (pstk.reshape(3, S, NPART, NL)
               .transpose(0, 2, 3, 1)                                # [3,128,NL,S]
               .reshape(3, NPART, NL * S)
               .transpose(1, 0, 2).reshape(NPART, 3 * NL * S))

        per_core.append({"par": np.ascontiguousarray(par),
                         "forc": np.ascontiguousarray(forc),
                         "pre": np.ascontiguousarray(pre)})
    return per_core


# --------------------------------------------------------------------------
# custom DVE ops
# --------------------------------------------------------------------------

def _register_custom_ops():
    from concourse import dve_ops
    from concourse.dve_ops import DveOp, OPS
    from concourse.dve_spec import Spec, Src0, Src1, relu, maxx, lower
    from concourse.dve_uop import DveOpSpec

    made = {}

    def reg(name, spec):
        for op in OPS:
            if op.name == name:
                made[name] = op
                return
        shas = {}
        for ver in ("v3", "v4"):
            uops = lower(spec, ver=ver)
            shas[ver] = DveOpSpec(name=name, opcode=0, uops=uops,
                                  rd1_en=True).sha(ver)
        op = DveOp(name, spec, subdim=False, uops_sha=shas)
        OPS.append(op)
        dve_ops.CUSTOM_DVE_SPECS[name] = spec
        dve_ops._SUB_OPCODE_FOR_NAME[name] = dve_ops._CUSTOM_DVE_ROW_BASE + len(OPS) - 1
        made[name] = op

    from concourse.dve_spec import C2
    reg("SUB_RELU_HYDRO", Spec(
        body=relu(Src0 - Src1),
        reference=lambda in0, in1, *a: np.maximum(in0 - in1, 0).astype(np.float32)))
    reg("SUB_MAXI_HYDRO", Spec(
        body=maxx(Src0 - Src1, C2),
        reference=lambda in0, in1, s0=0.0, s1=0.0, imm2=0.0:
            np.maximum(in0 - in1, imm2).astype(np.float32)))
    return made


# --------------------------------------------------------------------------
# device program
# --------------------------------------------------------------------------

def _build_program(outer_reps=1):
    import contextlib
    import concourse.bacc as bacc
    import concourse.mybir as mybir
    from concourse.tile import TileContext
    from concourse import bass

    ops = _register_custom_ops()
    SUB_RELU = ops["SUB_RELU_HYDRO"]
    SUB_MAXI = ops["SUB_MAXI_HYDRO"]

    dt = mybir.dt.float32
    AF = mybir.ActivationFunctionType
    OP = mybir.AluOpType

    # Force Ln+Exp to resolve to the combined 'natural_log_exp_and_others'
    # activation-table set: the greedy chooser otherwise alternates between
    # the exp-only and ln-only sets, paying a ~1us table reload twice per
    # time step. Strip Exp/Ln from every other set so only the combined set
    # (keeping its original act_info.json index) can satisfy them.
    import concourse.hw_specs as hw_specs
    if not getattr(bacc, "_hydro_act_patch", False):
        _orig_gat = bacc.get_activation_tables

        def _gat(arch):
            tabs = dict(_orig_gat(arch))
            EXP, LN = mybir.ActivationFunctionType.Exp, mybir.ActivationFunctionType.Ln
            if any(n == "natural_log_exp_and_others" and EXP in s and LN in s
                   for n, s in tabs.items()):
                for n in tabs:
                    if n != "natural_log_exp_and_others":
                        tabs[n] = tabs[n] - {EXP, LN}
            return tabs

        bacc.get_activation_tables = _gat
        bacc._hydro_act_patch = True

    nc = bacc.Bacc("TRN2", target_bir_lowering=False, debug=False,
                   num_devices=NCORES)

    d_par = nc.dram_tensor("par", [NPART, PAR_W], dt, kind="ExternalInput").ap()
    d_forc = nc.dram_tensor("forc", [NPART, S * 5 * NL], dt, kind="ExternalInput").ap()
    d_pre = nc.dram_tensor("pre", [NPART, 3 * NL * S], dt, kind="ExternalInput").ap()
    d_out = nc.dram_tensor("r_out", [NPART, 4 * S], dt, kind="ExternalOutput").ap()
    import os as _os
    _dbg = bool(_os.environ.get("HYDRO_DEBUG_DUMP"))
    d_dbg = {}
    if _dbg:
        for nm in ("PA", "Q0", "SUZ3", "QEB"):
            d_dbg[nm] = nc.dram_tensor("dbg_" + nm, [NPART, S * NL], dt,
                                       kind="ExternalOutput").ap()
        d_dbg["S1NB"] = nc.dram_tensor("dbg_S1NB", [NPART, (S + 1) * NL], dt,
                                       kind="ExternalOutput").ap()

    NZ = float(NEARZERO)

    def subrelu(out, a, b):
        nc.vector._custom_dve(SUB_RELU, out=out, in0=a, in1=b)

    def submaxi(out, a, b, imm):
        nc.vector._custom_dve(SUB_MAXI, out=out, in0=a, in1=b, imm2=imm)

    with TileContext(nc) as tc:
        rep_ctx = (tc.For_i(0, outer_reps, 1, name="outerrep")
                   if outer_reps > 1 else contextlib.nullcontext())
        with rep_ctx, tc.tile_pool(name="persist", bufs=1) as pp:
            par = pp.tile([NPART, PAR_W], dt, name="par", tag="par")
            nc.sync.dma_start(out=par[:, :], in_=d_par)

            def prm(name):
                i = PAR_NAMES.index(name)
                return par[:, i * NL:(i + 1) * NL]

            CWH, IFC, BETA, FC, ILPFC = (prm(n) for n in
                                         ("cwh", "ifc", "beta", "fc", "ilpfc"))
            PERC, UZL, K0, K1C = (prm(n) for n in ("perc", "uzl", "k0", "k1c"))
            SMAX, ISMX, NF, QMX = (prm(n) for n in ("smax", "ismx", "nf", "qmx"))

            def prm1(name, j):     # [P,1] per-partition scalar for lane j
                i = PAR_NAMES.index(name)
                return par[:, i * NL + j: i * NL + j + 1]

            # big sequence buffers (step-major: slot t*NL + j)
            SEQ = S * NL
            PA = pp.tile([NPART, SEQ], dt, name="PA", tag="PA")
            Q0 = pp.tile([NPART, SEQ], dt, name="Q0", tag="Q0")
            SUZ3 = pp.tile([NPART, SEQ], dt, name="SUZ3", tag="SUZ3")
            QEB = pp.tile([NPART, SEQ], dt, name="QEB", tag="QEB")
            S1NB = pp.tile([NPART, SEQ + NL], dt, name="S1NB", tag="S1NB")  # +1 init slot
            IN = pp.tile([NPART, SEQ], dt, name="IN", tag="IN")

            # states
            SP = pp.tile([NPART, NL], dt, name="SP", tag="SP")
            MW = pp.tile([NPART, NL], dt, name="MW", tag="MW")
            SM = pp.tile([NPART, NL], dt, name="SM", tag="SM")
            SUZ = pp.tile([NPART, NL], dt, name="SUZ", tag="SUZ")

            Z8 = pp.tile([NPART, NL], dt, name="Z8", tag="Z8")
            nc.vector.memset(Z8[:, :], 0.0)
            nc.vector.memset(SP[:, :], NZ)
            nc.vector.memset(MW[:, :], NZ)
            nc.vector.memset(SUZ[:, :], NZ)
            nc.vector.tensor_scalar(out=SM[:, :], in0=FC, scalar1=0.5,
                                    scalar2=None, op0=OP.mult)
            nc.vector.tensor_scalar(out=S1NB[:, 0:NL], in0=SMAX, scalar1=0.5,
                                    scalar2=None, op0=OP.mult)

            # ---------------- pre-pass: s0 scan + IN (per lane j) ----------
            with tc.tile_pool(name="pre", bufs=2) as prep:
                for j in range(NL):
                    cj = prep.tile([NPART, S], dt, name="cj", tag="cj")
                    bj = prep.tile([NPART, S], dt, name="bj", tag="bj")
                    ptj = prep.tile([NPART, S], dt, name="ptj", tag="ptj")
                    s0j = prep.tile([NPART, S], dt, name="s0j", tag="s0j")
                    nc.sync.dma_start(out=cj[:, :], in_=d_pre[:, 0 * NL * S + j * S: 0 * NL * S + (j + 1) * S])
                    nc.sync.dma_start(out=bj[:, :], in_=d_pre[:, 1 * NL * S + j * S: 1 * NL * S + (j + 1) * S])
                    nc.sync.dma_start(out=ptj[:, :], in_=d_pre[:, 2 * NL * S + j * S: 2 * NL * S + (j + 1) * S])
                    # s0' = max(s0 + c_t, b_t)
                    nc.vector.tensor_tensor_scan(out=s0j[:, :], data0=cj[:, :],
                                                 data1=bj[:, :], initial=NZ,
                                                 op0=OP.add, op1=OP.max)
                    INj = IN[:, j::NL]            # [128, S] strided lane view
                    # IN_t = ptot_t + s0_{t-1} - s0_t   (s0_{-1} = NZ)
                    nc.vector.tensor_tensor(out=INj, in0=ptj[:, :], in1=s0j[:, :],
                                            op=OP.subtract)
                    nc.vector.tensor_tensor(out=IN[:, NL + j::NL], in0=IN[:, NL + j::NL],
                                            in1=s0j[:, 0:S - 1], op=OP.add)
                    nc.vector.tensor_scalar(out=IN[:, j:j + 1], in0=IN[:, j:j + 1],
                                            scalar1=NZ, scalar2=None, op0=OP.add)

            # ---------------- main sequential loop -------------------------
            with tc.tile_pool(name="loop", bufs=3) as lp, \
                 tc.tile_pool(name="chunkp", bufs=1) as cp:
                ET = mybir.EngineType
                UH = U // 2 + 1          # 37 steps in first half
                with tc.For_i(0, NITER, 1,
                              hint_engines=(ET.DVE, ET.Activation, ET.SP)) as iv:
                    chunkA = cp.tile([NPART, UH * 5 * NL], dt, name="chunkA", tag="chunkA")
                    chunkB = cp.tile([NPART, (U - UH) * 5 * NL], dt, name="chunkB", tag="chunkB")
                    nc.sync.dma_start(out=chunkA[:, :],
                                      in_=d_forc[:, bass.ds(iv * (U * 5 * NL), UH * 5 * NL)])
                    nc.sync.dma_start(out=chunkB[:, :],
                                      in_=d_forc[:, bass.ds(iv * (U * 5 * NL) + UH * 5 * NL,
                                                            (U - UH) * 5 * NL)])

                    for s in range(U):
                        t = iv * U + s

                        def fr(f):    # forcing plane f at step s
                            if s < UH:
                                o = (s * 5 + f) * NL
                                return chunkA[:, o:o + NL]
                            o = ((s - UH) * 5 + f) * NL
                            return chunkB[:, o:o + NL]

                        SN, RU, RV, RAIN, PET = (fr(i) for i in range(5))

                        def tmp(tag):
                            return lp.tile([NPART, NL], dt, name=tag, tag=tag)[:, :]

                        def seq(buf, off=0):   # step slot of a sequence buffer
                            return buf[:, bass.ds((t + off) * NL, NL)]

                        tt_ = nc.vector.tensor_tensor
                        stt = nc.vector.scalar_tensor_tensor
                        import os as _os
                        _m = _os.environ.get("HYDRO_POOL_MODE", "both")
                        _suz_pool = _m in ("both", "suz")
                        _s1_pool = _m in ("both", "s1")

                        def mk(pool):
                            t = nc.gpsimd.tensor_tensor if pool else nc.vector.tensor_tensor
                            s = nc.gpsimd.tensor_scalar if pool else nc.vector.tensor_scalar
                            def sr(out, a, b):
                                t(out=out, in0=a, in1=b, op=OP.subtract)
                                s(out=out, in0=out, scalar1=0.0, scalar2=None, op0=OP.max)
                            return t, s, sr
                        ztt, zts, zsubrelu = mk(_suz_pool)      # SUZ block
                        ett, ets, esubrelu = mk(_s1_pool)       # S1 block

                        # HBV soil pow-chain first (ACT overlap with snow)
                        A = tmp("A"); tt_(out=A, in0=SM[:, :], in1=IFC, op=OP.mult)
                        LA = tmp("LA"); nc.scalar.activation(out=LA, in_=A, func=AF.Ln)
                        BL = tmp("BL"); tt_(out=BL, in0=BETA, in1=LA, op=OP.mult)
                        SW = tmp("SW"); nc.scalar.activation(out=SW, in_=BL, func=AF.Exp)

                        # HBV snow (DVE)
                        SPS = tmp("SPS"); tt_(out=SPS, in0=SP[:, :], in1=SN, op=OP.add)
                        SP2 = tmp("SP2"); subrelu(SP2, SPS, RU)
                        MELT = tmp("MELT"); tt_(out=MELT, in0=SPS, in1=SP2, op=OP.subtract)
                        MW1 = tmp("MW1"); tt_(out=MW1, in0=MW[:, :], in1=MELT, op=OP.add)
                        MW2 = tmp("MW2"); subrelu(MW2, MW1, RV)
                        REFR = tmp("REFR"); tt_(out=REFR, in0=MW1, in1=MW2, op=OP.subtract)
                        tt_(out=SP[:, :], in0=SP2, in1=REFR, op=OP.add)
                        CW = tmp("CW"); tt_(out=CW, in0=CWH, in1=SP[:, :], op=OP.mult)
                        TS2 = tmp("TS2"); subrelu(TS2, MW2, CW)
                        tt_(out=MW[:, :], in0=MW2, in1=TS2, op=OP.subtract)

                        # HBV soil (DVE)
                        RT = tmp("RT"); tt_(out=RT, in0=RAIN, in1=TS2, op=OP.add)
                        RECH = tmp("RECH")
                        stt(out=RECH, in0=SW, scalar=1.0, in1=RT, op0=OP.min, op1=OP.mult)
                        SM1 = tmp("SM1"); tt_(out=SM1, in0=SM[:, :], in1=RT, op=OP.add)
                        SM1B = tmp("SM1B"); tt_(out=SM1B, in0=SM1, in1=RECH, op=OP.subtract)
                        SM2 = tmp("SM2"); tt_(out=SM2, in0=SM1B, in1=FC, op=OP.min)
                        EXS = tmp("EXS"); subrelu(EXS, SM1B, FC)
                        B2 = tmp("B2"); tt_(out=B2, in0=SM2, in1=ILPFC, op=OP.mult)
                        ETR = tmp("ETR")
                        stt(out=ETR, in0=B2, scalar=1.0, in1=PET, op0=OP.min, op1=OP.mult)
                        submaxi(SM[:, :], SM2, ETR, NZ)

                        # HBV upper zone (GpSimd; min emulated via sub+relu)
                        RE = tmp("RE"); tt_(out=RE, in0=RECH, in1=EXS, op=OP.add)
                        SUZ1 = tmp("SUZ1"); ztt(out=SUZ1, in0=SUZ[:, :], in1=RE, op=OP.add)
                        SUZ2 = tmp("SUZ2"); zsubrelu(SUZ2, SUZ1, PERC)
                        ztt(out=seq(PA), in0=SUZ1, in1=SUZ2, op=OP.subtract)
                        UZ = tmp("UZ"); zsubrelu(UZ, SUZ2, UZL)
                        ztt(out=seq(Q0), in0=K0, in1=UZ, op=OP.mult)
                        ztt(out=seq(SUZ3), in0=SUZ2, in1=seq(Q0), op=OP.subtract)
                        ztt(out=SUZ[:, :], in0=K1C, in1=seq(SUZ3), op=OP.mult)

                        # EXP s1 (GpSimd + ACT; state lives in S1NB slots).
                        # NOTE: Pool tensor_scalar must never target a ds()
                        # slot (HW miscompiles) - relu into plain tmps; ds
                        # stores only via Pool tensor_tensor. qspill+qb are
                        # blended into QE in-loop (fused store).
                        S1At = tmp("S1At")
                        ett(out=S1At, in0=S1NB[:, bass.ds(t * NL, NL)],
                            in1=IN[:, bass.ds(t * NL, NL)], op=OP.add)
                        QSPt = tmp("QSPt"); esubrelu(QSPt, S1At, SMAX)
                        S1C = tmp("S1C"); ett(out=S1C, in0=S1At, in1=QSPt, op=OP.subtract)
                        B2e = tmp("B2e"); ett(out=B2e, in0=S1C, in1=ISMX, op=OP.mult)
                        ets(out=B2e, in0=B2e, scalar1=1.0, scalar2=None, op0=OP.min)
                        ETR2 = tmp("ETR2"); ett(out=ETR2, in0=B2e, in1=PET, op=OP.mult)
                        S1Dt = tmp("S1Dt"); esubrelu(S1Dt, S1C, ETR2)
                        D = tmp("D"); tt_(out=D, in0=SMAX, in1=S1Dt, op=OP.subtract)
                        E = tmp("E"); tt_(out=E, in0=NF, in1=D, op=OP.mult)
                        EE = tmp("EE"); nc.scalar.activation(out=EE, in_=E, func=AF.Exp)
                        QB0 = tmp("QB0"); ett(out=QB0, in0=QMX, in1=EE, op=OP.mult)
                        S1Nt = tmp("S1Nt"); esubrelu(S1Nt, S1Dt, QB0)
                        ett(out=S1NB[:, bass.ds((t + 1) * NL, NL)], in0=S1Nt,
                            in1=Z8[:, :], op=OP.add)
                        QBt = tmp("QBt"); ett(out=QBt, in0=S1Dt, in1=S1Nt, op=OP.subtract)
                        ett(out=seq(QEB), in0=QSPt, in1=QBt, op=OP.add)

            if _dbg:
                for nm, buf in (("PA", PA), ("Q0", Q0), ("SUZ3", SUZ3),
                                ("QEB", QEB), ("S1NB", S1NB)):
                    nc.sync.dma_start(out=d_dbg[nm], in_=buf[:, :])

            # ---------------- post-pass ------------------------------------
            with tc.tile_pool(name="post", bufs=2) as po:
                tt_ = nc.vector.tensor_tensor
                stt = nc.vector.scalar_tensor_tensor

                ZERO = po.tile([NPART, S], dt, name="zero", tag="zero")
                nc.vector.memset(ZERO[:, :], 0.0)

                # q1 fold: Q0_j += k1_j * SUZ3_j
                for j in range(NL):
                    stt(out=Q0[:, j::NL], in0=SUZ3[:, j::NL], scalar=prm1("k1", j),
                        in1=Q0[:, j::NL], op0=OP.mult, op1=OP.add)

                # slz scan + q2 fold
                for j in range(NL):
                    pac = po.tile([NPART, S], dt, name="pac", tag="pac")
                    k2cj = po.tile([NPART, S], dt, name="k2cj", tag="k2cj")
                    slzs = po.tile([NPART, S], dt, name="slzs", tag="slzs")
                    nc.vector.tensor_copy(out=pac[:, :], in_=PA[:, j::NL])
                    nc.vector.tensor_scalar(out=k2cj[:, :], in0=ZERO[:, :],
                                            scalar1=prm1("k2c", j), scalar2=None,
                                            op0=OP.add)
                    # slz' = (pa_t + slz) * k2c
                    nc.vector.tensor_tensor_scan(out=slzs[:, :], data0=pac[:, :],
                                                 data1=k2cj[:, :], initial=NZ,
                                                 op0=OP.add, op1=OP.mult)
                    # SZ1 (into PA): pa_t += slz_{t-1};  pa_0 += NZ
                    tt_(out=PA[:, NL + j::NL], in0=PA[:, NL + j::NL],
                        in1=slzs[:, 0:S - 1], op=OP.add)
                    nc.vector.tensor_scalar(out=PA[:, j:j + 1], in0=PA[:, j:j + 1],
                                            scalar1=NZ, scalar2=None, op0=OP.add)
                    # q2 = SZ1 - slz'
                    tt_(out=slzs[:, :], in0=PA[:, j::NL], in1=slzs[:, :], op=OP.subtract)
                    tt_(out=Q0[:, j::NL], in0=Q0[:, j::NL], in1=slzs[:, :], op=OP.add)

                # blend: QHE = QH + QE (QE already assembled in-loop)
                tt_(out=Q0[:, :], in0=Q0[:, :], in1=QEB[:, :], op=OP.add)      # QHE

                # blend over NMUL -> QS [128, 4*QSW] (lane-major, 14 zero pad)
                QS = pp.tile([NPART, 4 * QSW], dt, name="QS", tag="QS")
                nc.vector.memset(QS[:, :], 0.0)
                for jp in range(4):
                    tt_(out=QS[:, jp * QSW + QSPAD: jp * QSW + QSW],
                        in0=Q0[:, 2 * jp::NL], in1=Q0[:, 2 * jp + 1::NL], op=OP.add)

                # routing: R[jp, t] = sum_k w4[k, jp] * QS[jp, 14 + t - k]
                R = pp.tile([NPART, 4 * S], dt, name="R", tag="R")
                nc.vector.memset(R[:, :], 0.0)
                for jp in range(4):
                    rj = R[:, jp * S:(jp + 1) * S]
                    for k in range(LENF):
                        qsh = QS[:, jp * QSW + QSPAD - k: jp * QSW + QSPAD - k + S]
                        wk = par[:, W4_OFF + k * 4 + jp: W4_OFF + k * 4 + jp + 1]
                        stt(out=rj, in0=qsh, scalar=wk, in1=rj, op0=OP.mult, op1=OP.add)

                nc.sync.dma_start(out=d_out, in_=R[:, :])

    nc.compile()
    return nc


_PROGRAM = None


def _get_program():
    global _PROGRAM
    if _PROGRAM is None:
        _PROGRAM = _build_program()
    return _PROGRAM


def kernel(x, raw_phy_static, _trace=False):
    from concourse.bass_utils import run_bass_kernel_spmd

    per_core = _host_prepare(x, raw_phy_static)
    nc = _get_program()
    res = run_bass_kernel_spmd(nc, per_core, core_ids=list(range(NCORES)),
                               trace=_trace)
    out = np.empty((S, G), f32)
    for d in range(NCORES):
        r = res.results[d]["r_out"].reshape(NPART, 4, S)   # [p, jp, t]
        # basin b = 4p + jp
        rb = r.transpose(2, 0, 1).reshape(S, NPART * 4)    # [t, b]
        out[:, d * GPC:(d + 1) * GPC] = rb[:, :GPC]
    if _trace:
        return out, res
    return out

